# revision 1
# baseline (speedup 1.0000x reference)
"""Trainium2 Bass kernel for nn_LmLSTM: embedding -> 2x masked LSTM -> vocab projection.

Sharding: the LSTM recurrence is gate-sharded across the 8 cores (core r owns
hidden slice [128r,128r+128) of both layers and computes the i/f/g/o gates for
that slice each step); the full hidden state is re-assembled every step with an
AllGather of the 8 bf16 h-shards. The [H,V] output projection is sharded over
vocab (4000 cols/core) and streamed from DRAM.

All activations live transposed ([feature, token] with features on partitions)
so gate nonlinearities/state updates run on full 128-partition tiles.
"""

import os
import sys
import types
import contextlib
import ctypes

import numpy as np
import ml_dtypes

# ---------------------------------------------------------------------------
# Environment shims (self-contained): NTFF profile hook + walrus wait-split.
# ---------------------------------------------------------------------------


def _install_axon_profile_hook():
    if "antenv.axon_hooks" in sys.modules:
        return
    holder = [None]
    mod = types.ModuleType("antenv.axon_hooks")
    mod.set_axon_ntff_profile_hook = lambda h: holder.__setitem__(0, h)
    mod.get_axon_ntff_profile_hook = lambda: holder[0]
    sys.modules["antenv.axon_hooks"] = mod
    try:
        import antenv

        antenv.axon_hooks = mod
        from trn_agent_boot.trn_boot import _ntff_profile_via_ctypes

        mod.set_axon_ntff_profile_hook(
            _ntff_profile_via_ctypes("/opt/axon/libaxon_pjrt.so")
        )
    except Exception:
        pass


_install_axon_profile_hook()

import concourse.bass as bass  # noqa: E402
import concourse.mybir as mybir  # noqa: E402
import concourse.tile as tile  # noqa: E402
from concourse.bass_utils import run_bass_kernel_spmd  # noqa: E402


def _install_wait_split():
    """This container's walrus accepts at most one sem-wait per instruction.
    Hoist excess waits onto same-engine nops placed just before."""
    if getattr(bass.Bass, "_waitsplit_installed", False):
        return
    counter = [0]

    def _split(m):
        for f in m.functions:
            for bb in f.blocks:
                il = bb.instructions
                if not any(
                    i.sync_info is not None and len(i.sync_info.on_wait) > 1
                    for i in il
                ):
                    continue
                new = []
                for inst in il:
                    si = inst.sync_info
                    if si is not None and len(si.on_wait) > 1:
                        waits = list(si.on_wait)
                        si.on_wait = waits[:1]
                        for w in waits[1:]:
                            counter[0] += 1
                            nop = mybir.InstNoOp(
                                name=f"waitsplit_{counter[0]}", ins=[], outs=[]
                            )
                            nop.engine = inst.engine
                            nop.sync_info = mybir.SyncInfo(
                                on_wait=[w], on_update=[]
                            )
                            new.append(nop)
                    new.append(inst)
                il.clear()
                il.extend(new)

    orig = bass.Bass.to_json_bytes

    def patched(self, *a, **kw):
        _split(self.m)
        return orig(self, *a, **kw)

    bass.Bass.to_json_bytes = patched
    bass.Bass._waitsplit_installed = True


_install_wait_split()

# ---------------------------------------------------------------------------
# Problem constants
# ---------------------------------------------------------------------------
V, E, H = 32000, 512, 1024
B = 16
T = int(os.environ.get("KERNEL_T", "256"))
NC = 8
VS = V // NC  # 4000 vocab cols per core
NTOK = B * T
NTC = NTOK // 128  # token chunks
F32 = mybir.dt.float32
BF16 = mybir.dt.bfloat16
SIG = mybir.ActivationFunctionType.Sigmoid
TANH = mybir.ActivationFunctionType.Tanh


def build_nc():
    nc = bass.Bass()
    d_w0 = nc.dram_tensor("w0p", [128, 12 * 4 * 128], BF16, kind="ExternalInput")
    d_w1 = nc.dram_tensor("w1p", [128, 16 * 4 * 128], BF16, kind="ExternalInput")
    d_wout = nc.dram_tensor("woutp", [128, 8 * VS], BF16, kind="ExternalInput")
    d_b0 = nc.dram_tensor("b0t", [128, 4], F32, kind="ExternalInput")
    d_b1 = nc.dram_tensor("b1t", [128, 4], F32, kind="ExternalInput")
    d_bout = nc.dram_tensor("boutb", [128, VS], F32, kind="ExternalInput")
    d_xt = nc.dram_tensor("xt", [E, T * B], BF16, kind="ExternalInput")
    d_maskb = nc.dram_tensor("maskb", [128, T * B], mybir.dt.uint8, kind="ExternalInput")
    d_mpt = nc.dram_tensor("maskpt", [128, NTC], F32, kind="ExternalInput")
    d_impt = nc.dram_tensor("invmpt", [128, NTC], F32, kind="ExternalInput")
    d_out = nc.dram_tensor("out", [NTOK, VS], F32, kind="ExternalOutput")

    rg = [list(range(NC))]

    with tile.TileContext(nc) as tc:
        with (
            tc.tile_pool(name="wp", bufs=1) as wp,
            tc.tile_pool(name="sp", bufs=3) as sp,
            tc.tile_pool(name="pp", bufs=2, space="PSUM") as pp,
            tc.tile_pool(name="dp", bufs=3, space="DRAM") as dp,
        ):
            # ---- persistent loads ----
            w0t = wp.tile([128, 12 * 4 * 128], BF16, tag="w0t")
            w1t = wp.tile([128, 16 * 4 * 128], BF16, tag="w1t")
            b0t = wp.tile([128, 4], F32, tag="b0t")
            b1t = wp.tile([128, 4], F32, tag="b1t")
            boutb = wp.tile([128, VS], F32, tag="boutb")
            maskb = wp.tile([128, T * B], mybir.dt.uint8, tag="maskb")
            mpt = wp.tile([128, NTC], F32, tag="mpt")
            impt = wp.tile([128, NTC], F32, tag="impt")
            nc.gpsimd.dma_start(w0t[:], d_w0[:])
            nc.gpsimd.dma_start(w1t[:], d_w1[:])
            nc.gpsimd.dma_start(b0t[:], d_b0[:])
            nc.gpsimd.dma_start(b1t[:], d_b1[:])
            nc.gpsimd.dma_start(boutb[:], d_bout[:])
            nc.gpsimd.dma_start(maskb[:], d_maskb[:])
            nc.gpsimd.dma_start(mpt[:], d_mpt[:])
            nc.gpsimd.dma_start(impt[:], d_impt[:])
            xt = []
            for k in range(4):
                xk = wp.tile([128, T * B], BF16, tag=f"xt{k}")
                nc.gpsimd.dma_start(xk[:], d_xt[128 * k : 128 * (k + 1), :])
                xt.append(xk)
            hist = []
            for k in range(NC):
                hk = wp.tile([128, NTOK], BF16, tag=f"hist{k}")
                hist.append(hk)

            # ---- persistent state ----
            c0 = wp.tile([128, 16], F32, tag="c0")
            c1 = wp.tile([128, 16], F32, tag="c1")
            h0sh = wp.tile([128, 16], F32, tag="h0sh")
            h1sh = wp.tile([128, 16], F32, tag="h1sh")
            h0bf = wp.tile([128, 16], BF16, tag="h0bf")
            h1bf = wp.tile([128, 16], BF16, tag="h1bf")
            for t_ in (c0, c1, h0sh, h1sh, h0bf, h1bf):
                nc.vector.memset(t_[:], 0.0)
            zfull = wp.tile([128, 128], BF16, tag="zfull")
            nc.vector.memset(zfull[:], 0.0)
            h0full, h1full = zfull, zfull

            def lstm_half(zps, gt, tmp, cstate, hstate, hbf, btile, mslice):
                # gates: [i|f|g|o] each 16 cols of zps
                nc.scalar.activation(gt[:, 0:16], zps[:, 0:16], SIG, bias=btile[:, 0:1])
                nc.scalar.activation(gt[:, 16:32], zps[:, 16:32], SIG, bias=btile[:, 1:2])
                nc.scalar.activation(gt[:, 32:48], zps[:, 32:48], TANH, bias=btile[:, 2:3])
                nc.scalar.activation(gt[:, 48:64], zps[:, 48:64], SIG, bias=btile[:, 3:4])
                nc.vector.tensor_mul(tmp[:, 0:16], gt[:, 0:16], gt[:, 32:48])  # i*g
                nc.vector.tensor_mul(tmp[:, 16:32], gt[:, 16:32], cstate[:])  # f*c
                nc.vector.tensor_add(tmp[:, 32:48], tmp[:, 0:16], tmp[:, 16:32])  # cn
                nc.scalar.activation(tmp[:, 48:64], tmp[:, 32:48], TANH)  # tanh(cn)
                nc.vector.tensor_mul(tmp[:, 64:80], gt[:, 48:64], tmp[:, 48:64])  # hn
                nc.vector.copy_predicated(cstate[:], mslice, tmp[:, 32:48])
                nc.vector.copy_predicated(hstate[:], mslice, tmp[:, 64:80])
                nc.vector.tensor_copy(hbf[:], hstate[:])  # cast f32->bf16

            for t in range(T + 1):
                if t < T:
                    z0 = pp.tile([128, 64], F32, tag="z0")
                    for gc in range(4):
                        for k in range(12):
                            rhs = (
                                xt[k][:, 16 * t : 16 * t + 16]
                                if k < 4
                                else h0full[:, 16 * (k - 4) : 16 * (k - 4) + 16]
                            )
                            nc.tensor.matmul(
                                z0[:, 16 * gc : 16 * gc + 16],
                                w0t[:, (k * 4 + gc) * 128 : (k * 4 + gc) * 128 + 128],
                                rhs,
                                start=(k == 0),
                                stop=(k == 11),
                            )
                    g0 = sp.tile([128, 64], F32, tag="g0")
                    tmp0 = sp.tile([128, 80], F32, tag="tmp0")
                    lstm_half(
                        z0, g0, tmp0, c0, h0sh, h0bf, b0t,
                        maskb[:, 16 * t : 16 * t + 16],
                    )
                if t >= 1:
                    s = t - 1
                    z1 = pp.tile([128, 64], F32, tag="z1")
                    for gc in range(4):
                        for k in range(16):
                            rhs = (
                                h0full[:, 16 * k : 16 * k + 16]
                                if k < 8
                                else h1full[:, 16 * (k - 8) : 16 * (k - 8) + 16]
                            )
                            nc.tensor.matmul(
                                z1[:, 16 * gc : 16 * gc + 16],
                                w1t[:, (k * 4 + gc) * 128 : (k * 4 + gc) * 128 + 128],
                                rhs,
                                start=(k == 0),
                                stop=(k == 15),
                            )
                    g1 = sp.tile([128, 64], F32, tag="g1")
                    tmp1 = sp.tile([128, 80], F32, tag="tmp1")
                    lstm_half(
                        z1, g1, tmp1, c1, h1sh, h1bf, b1t,
                        maskb[:, 16 * s : 16 * s + 16],
                    )

                # ---- exchange both shards ----
                cc_in = dp.tile([256, 16], BF16, tag="cc_in")
                cc_out = dp.tile([2048, 16], BF16, tag="cc_out")
                nc.gpsimd.dma_start(cc_in[0:128, :], h0bf[:])
                nc.gpsimd.dma_start(cc_in[128:256, :], h1bf[:])
                nc.gpsimd.collective_compute(
                    "AllGather",
                    mybir.AluOpType.bypass,
                    ins=[cc_in.opt()],
                    outs=[cc_out.opt()],
                    replica_groups=rg,
                )
                h0new = sp.tile([128, 128], BF16, tag="h0full")
                h1new = sp.tile([128, 128], BF16, tag="h1full")
                v4 = cc_out.rearrange("(k s p) b -> s p k b", s=2, p=128)
                nc.gpsimd.dma_start(
                    h0new.rearrange("p (k b) -> p k b", k=8), v4[0]
                )
                nc.gpsimd.dma_start(
                    h1new.rearrange("p (k b) -> p k b", k=8), v4[1]
                )
                if t >= 1:
                    s = t - 1
                    for k in range(NC):
                        dst = hist[k].rearrange("p (b t) -> p b t", t=T)[:, :, s]
                        nc.vector.tensor_copy(dst, h1new[:, 16 * k : 16 * k + 16])
                h0full, h1full = h0new, h1new

            # ---- projection: logits[tok, VS] = h1 @ Wout_shard + bout ----
            for n in range(8):
                nsz = VS // 8  # 500
                wtiles = []
                for k in range(8):
                    wt = sp.tile([128, nsz], BF16, tag=f"wout{k}")
                    nc.gpsimd.dma_start(
                        wt[:], d_wout[:, k * VS + n * nsz : k * VS + (n + 1) * nsz]
                    )
                    wtiles.append(wt)
                for tc_ in range(NTC):
                    ps = pp.tile([128, nsz], F32, tag="proj")
                    for k in range(8):
                        nc.tensor.matmul(
                            ps[:],
                            hist[k][:, 128 * tc_ : 128 * (tc_ + 1)],
                            wtiles[k][:],
                            start=(k == 0),
                            stop=(k == 7),
                        )
                    lg = sp.tile([128, nsz], F32, tag="lg")
                    nc.vector.tensor_add(lg[:], ps[:], boutb[:, n * nsz : (n + 1) * nsz])
                    nc.vector.tensor_scalar_mul(lg[:], lg[:], mpt[:, tc_ : tc_ + 1])
                    if n == 0:
                        nc.vector.tensor_add(
                            lg[:, 0:1], lg[:, 0:1], impt[:, tc_ : tc_ + 1]
                        )
                    nc.gpsimd.dma_start(
                        d_out[128 * tc_ : 128 * (tc_ + 1), n * nsz : (n + 1) * nsz],
                        lg[:],
                    )
    return nc


_NC_CACHE = [None]


def kernel(tokens, emb, Wx0, Wh0, b0, Wx1, Wh1, b1, Wout, bout):
    tokens = np.asarray(tokens)
    toks = tokens.astype(np.int64)
    emb = np.asarray(emb, np.float32)
    fm = (toks != 0).astype(np.float32)  # [B,T]

    x = emb[toks]  # [B,T,E]
    xt = np.ascontiguousarray(x[:, :T].transpose(2, 1, 0).reshape(E, T * B))
    xt = xt.astype(ml_dtypes.bfloat16)

    fm = fm[:, :T]
    maskb = np.broadcast_to(
        np.ascontiguousarray(fm.T).reshape(1, T * B), (128, T * B)
    ).astype(np.uint8)
    fm_flat = fm.reshape(-1)  # (b,t) order
    mpt = np.ascontiguousarray(fm_flat.reshape(NTC, 128).T).astype(np.float32)
    impt_base = np.ascontiguousarray((1.0 - fm_flat).reshape(NTC, 128).T).astype(
        np.float32
    )

    def pack(w, nk):
        # w: [nk*128, 512] -> [128, nk*4*128]
        a = np.asarray(w, np.float32).reshape(nk, 128, 4, 128)
        return np.ascontiguousarray(a.transpose(1, 0, 2, 3)).reshape(
            128, nk * 4 * 128
        ).astype(ml_dtypes.bfloat16)

    in_maps = []
    for r in range(NC):
        cols = np.concatenate(
            [g * H + np.arange(128 * r, 128 * (r + 1)) for g in range(4)]
        )
        w0 = np.concatenate([np.asarray(Wx0)[:, cols], np.asarray(Wh0)[:, cols]], 0)
        w1 = np.concatenate([np.asarray(Wx1)[:, cols], np.asarray(Wh1)[:, cols]], 0)
        wo = np.asarray(Wout, np.float32)[:, VS * r : VS * (r + 1)]  # [1024, VS]
        woutp = (
            np.ascontiguousarray(wo.reshape(8, 128, VS).transpose(1, 0, 2))
            .reshape(128, 8 * VS)
            .astype(ml_dtypes.bfloat16)
        )
        in_maps.append(
            {
                "w0p": pack(w0, 12),
                "w1p": pack(w1, 16),
                "woutp": woutp,
                "b0t": np.ascontiguousarray(
                    np.asarray(b0, np.float32)[cols].reshape(4, 128).T
                ),
                "b1t": np.ascontiguousarray(
                    np.asarray(b1, np.float32)[cols].reshape(4, 128).T
                ),
                "boutb": np.broadcast_to(
                    np.asarray(bout, np.float32)[VS * r : VS * (r + 1)], (128, VS)
                ).copy(),
                "xt": xt,
                "maskb": maskb,
                "maskpt": mpt,
                "invmpt": impt_base if r == 0 else np.zeros_like(impt_base),
            }
        )

    if _NC_CACHE[0] is None:
        _NC_CACHE[0] = build_nc()
    nc = _NC_CACHE[0]

    trace = os.environ.get("KERNEL_TRACE", "0") == "1"
    res = run_bass_kernel_spmd(
        nc, in_maps, core_ids=list(range(NC)), trace=trace
    )
    if trace and res.exec_time_ns is not None:
        print(f"HW exec time: {res.exec_time_ns} ns")

    logits = np.concatenate(
        [res.results[r]["out"] for r in range(NC)], axis=1
    )  # [T*B? no: [NTOK,(b,t)] x V]
    out = logits.reshape(B, T, V).astype(np.float32)
    if T < tokens.shape[1]:
        full = np.zeros((B, tokens.shape[1], V), np.float32)
        full[:, :T] = out
        out = full
    return out

